# revision 3
# baseline (speedup 1.0000x reference)
"""BoundaryLoss Trainium2 kernel (8-core data-parallel).

Math (equivalent to the reference for inputs whose boundary dilation covers
the image, which is checked on-device via a flag and falls back otherwise):

  boundary b[p] = 1 iff the 3x3 window around p is NOT single-class
                = (window-sum of adjacent-pixel class-difference indicators) > 0
  If dilate3x3(b) is all-ones, the capped distance transform gives
  weights = c1 + (1-c1)*b  with c1 = exp(-1/theta), so
  loss = [ c1*(Σlse - Σx_t) + (1-c1)*(Σb*lse - Σb*x_t) ] / N
  where lse = logsumexp_c(x), x_t = logit at target class.

Per-core device computation (2 images/core, strips of 128 rows):
  - eh/ev: horizontal/vertical class-difference indicators  (DVE, bf16)
  - window sums: horizontal adds on DVE; vertical via banded-matrix matmuls
    accumulated in PSUM (PE); b = sum > 0
  - dilation flag: 3x3 window-sum of b via PE bands; min-reduce per strip
  - CE: exp on ACT (bf16), plane-sum via identity-matmul PSUM accumulation
    (PE), lse = Ln on ACT with free-dim sum accumulator
  - Σx_t / Σb*x_t: per-class scalar_tensor_tensor (mask-select-multiply with
    row-sum accumulator) on DVE; the b-weighted variant uses a masked target
    t'' = t + 255*b so non-boundary pixels never match a class.
Host sums the per-strip accumulator columns in float64 and applies the
closed form; if any core's flag shows an uncovered pixel, recompute exactly
on host (numpy port of the reference; statistically never taken).
"""
import math
import numpy as np
import ml_dtypes
import concourse.bass as bass
import concourse.tile as tile
from concourse import mybir
from concourse.bass_utils import run_bass_kernel_spmd

BF16 = mybir.dt.bfloat16
F32 = mybir.dt.float32
AF = mybir.ActivationFunctionType
OP = mybir.AluOpType
AX = mybir.AxisListType

B, C, H, W = 16, 8, 512, 512
N_CORES = 8
PER = B // N_CORES            # images per core
S = H // 128                  # strips per image
NSTRIP = PER * S              # strips per core
THETA = 5.0
MAX_ITERS = 15
C1 = math.exp(-1.0 / THETA)
NPIX = B * H * W

# accumulator column layout per strip j: base = j*19
#  +0 lse_sum, +1 blse_sum, +2 flag_min, +3..10 xt[c], +11..18 bxt[c]
COLS_PER_STRIP = 19
NCOLS = NSTRIP * COLS_PER_STRIP


def _split_sync_waits(nc, max_waits=1):
    """Walrus CoreV3 codegen rejects >1 sync wait per instruction; hoist
    extras onto NoOps inserted just before."""
    k = 0
    for f in nc.m.functions:
        for bb in f.blocks:
            new = []
            for ins in bb.instructions:
                w = list(ins.sync_info.on_wait) if ins.sync_info else []
                if len(w) > max_waits:
                    extra, keep = w[:-max_waits], w[-max_waits:]
                    for s0 in range(0, len(extra), max_waits):
                        nop = mybir.InstNoOp(
                            name=f"I-wsplit-{k}", ins=[], outs=[],
                            sync_info=mybir.SyncInfo(
                                on_wait=extra[s0:s0 + max_waits], on_update=[]),
                            engine=ins.engine)
                        k += 1
                        new.append(nop)
                    ins.sync_info.on_wait = keep
                new.append(ins)
            bb.instructions = new


def _band_consts():
    """bf16 [128, 5*128]: T3 (tridiag), T2 (k in {p-1,p}), U (k=127 -> p=0),
    D (k=0 -> p=127), I (identity). lhsT layout: [k, p]."""
    k = np.arange(128)[:, None]
    p = np.arange(128)[None, :]
    T3 = (np.abs(k - p) <= 1).astype(np.float32)
    T2 = ((k == p) | (k == p - 1)).astype(np.float32)
    U = ((k == 127) & (p == 0)).astype(np.float32)
    D = ((k == 0) & (p == 127)).astype(np.float32)
    I = (k == p).astype(np.float32)
    return np.concatenate([T3, T2, U, D, I], axis=1).astype(ml_dtypes.bfloat16)


_NC_CACHE = {}


def _build_nc():
    if "nc" in _NC_CACHE:
        return _NC_CACHE["nc"]
    nc = bass.Bass()
    xl = nc.dram_tensor("xl", [PER, C, H, W], BF16, kind="ExternalInput")
    tg = nc.dram_tensor("tg", [PER, H, W], BF16, kind="ExternalInput")
    cst = nc.dram_tensor("cst", [128, 5 * 128], BF16, kind="ExternalInput")
    out = nc.dram_tensor("out", [128, NCOLS], F32, kind="ExternalOutput")

    with tile.TileContext(nc) as tc:
        with (
            tc.tile_pool(name="pc", bufs=1) as pc,
            tc.tile_pool(name="pp", bufs=2) as pp,      # persistent per-strip maps
            tc.tile_pool(name="pt", bufs=3) as pt,      # transients
            tc.tile_pool(name="px", bufs=3) as px,      # big x tiles
            tc.tile_pool(name="pa", bufs=1) as pa,      # accumulator columns
            tc.tile_pool(name="ps", bufs=2, space="PSUM") as ps,
        ):
            cons = pc.tile([128, 5 * 128], BF16, tag="cons")
            nc.sync.dma_start(cons[:], cst[:])
            T3 = cons[:, 0:128]
            T2 = cons[:, 128:256]
            Uc = cons[:, 256:384]
            Dc = cons[:, 384:512]
            Ic = cons[:, 512:640]

            cols = pa.tile([128, NCOLS], F32, tag="cols")

            for img in range(PER):
                t_t, h2_t, h3_t, b_t, h3b_t = [], [], [], [], []
                # ---- P1: load t, edge maps, horizontal window sums ----
                for s in range(S):
                    r0 = s * 128
                    t = pp.tile([128, W], BF16, tag=f"t{s}")
                    nc.gpsimd.dma_start(t[:], tg[img, r0:r0 + 128, :])
                    td = pt.tile([128, W], BF16, tag="td")
                    if s < S - 1:
                        nc.gpsimd.dma_start(td[:], tg[img, r0 + 1:r0 + 129, :])
                    else:
                        nc.gpsimd.dma_start(td[0:127, :], tg[img, r0 + 1:r0 + 128, :])
                        nc.gpsimd.dma_start(td[127:128, :], tg[img, H - 1:H, :])
                    # eh[c] = t[c] != t[c+1], c<511; col 511 = 0
                    eh = pt.tile([128, W], BF16, tag="eh")
                    nc.gpsimd.memset(eh[:, W - 1:W], 0.0)
                    nc.vector.tensor_tensor(out=eh[:, 0:W - 1], in0=t[:, 0:W - 1],
                                            in1=t[:, 1:W], op=OP.not_equal)
                    # ev = t != t_down   (row 511 clamps to itself -> 0)
                    ev = pt.tile([128, W], BF16, tag="ev")
                    nc.vector.tensor_tensor(out=ev[:], in0=t[:], in1=td[:],
                                            op=OP.not_equal)
                    # H2eh[c] = eh[c-1] + eh[c]
                    h2 = pp.tile([128, W], BF16, tag=f"h2{s}")
                    nc.gpsimd.tensor_copy(h2[:, 0:1], eh[:, 0:1])
                    nc.vector.tensor_tensor(out=h2[:, 1:W], in0=eh[:, 0:W - 1],
                                            in1=eh[:, 1:W], op=OP.add)
                    # H3ev[c] = ev[c-1] + ev[c] + ev[c+1]
                    h3 = pp.tile([128, W], BF16, tag=f"h3{s}")
                    tmp = pt.tile([128, W], BF16, tag="tmp")
                    nc.vector.tensor_tensor(out=tmp[:, 0:W - 1], in0=ev[:, 0:W - 1],
                                            in1=ev[:, 1:W], op=OP.add)
                    nc.vector.tensor_tensor(out=h3[:, 1:W - 1], in0=tmp[:, 0:W - 2],
                                            in1=ev[:, 2:W], op=OP.add)
                    nc.gpsimd.tensor_copy(h3[:, 0:1], tmp[:, 0:1])
                    nc.gpsimd.tensor_copy(h3[:, W - 1:W], tmp[:, W - 2:W - 1])
                    t_t.append(t); h2_t.append(h2); h3_t.append(h3)

                # ---- P2: vertical sums via PE bands -> b; then H3 of b ----
                for s in range(S):
                    sb = ps.tile([128, W], F32, tag="sb")
                    nc.tensor.matmul(sb[:], T3, h2_t[s][:], start=True, stop=False)
                    if s > 0:
                        nc.tensor.matmul(sb[:], Uc, h2_t[s - 1][:], start=False, stop=False)
                    if s < S - 1:
                        nc.tensor.matmul(sb[:], Dc, h2_t[s + 1][:], start=False, stop=False)
                    nc.tensor.matmul(sb[:], T2, h3_t[s][:], start=False, stop=(s == 0))
                    if s > 0:
                        nc.tensor.matmul(sb[:], Uc, h3_t[s - 1][:], start=False, stop=True)
                    b = pp.tile([128, W], BF16, tag=f"b{s}")
                    nc.vector.tensor_scalar(out=b[:], in0=sb[:], scalar1=0.5,
                                            scalar2=None, op0=OP.is_gt)
                    # H3b
                    h3b = pp.tile([128, W], BF16, tag=f"h3b{s}")
                    tmp2 = pt.tile([128, W], BF16, tag="tmp2")
                    nc.vector.tensor_tensor(out=tmp2[:, 0:W - 1], in0=b[:, 0:W - 1],
                                            in1=b[:, 1:W], op=OP.add)
                    nc.vector.tensor_tensor(out=h3b[:, 1:W - 1], in0=tmp2[:, 0:W - 2],
                                            in1=b[:, 2:W], op=OP.add)
                    nc.gpsimd.tensor_copy(h3b[:, 0:1], tmp2[:, 0:1])
                    nc.gpsimd.tensor_copy(h3b[:, W - 1:W], tmp2[:, W - 2:W - 1])
                    b_t.append(b); h3b_t.append(h3b)

                # ---- P3a: dilation flag + t'' + exp + plane sums ----
                e_t, lse_t, tpp_t = [], [], []
                for s in range(S):
                    j = img * S + s
                    base = j * COLS_PER_STRIP
                    sd = ps.tile([128, W], F32, tag="sd")
                    nc.tensor.matmul(sd[:], T3, h3b_t[s][:], start=True,
                                     stop=(s == 0 and S == 1))
                    if s > 0:
                        nc.tensor.matmul(sd[:], Uc, h3b_t[s - 1][:], start=False,
                                         stop=(s == S - 1))
                    if s < S - 1:
                        nc.tensor.matmul(sd[:], Dc, h3b_t[s + 1][:], start=False,
                                         stop=True)
                    # min over dilation sums; host checks > 0
                    nc.vector.tensor_reduce(out=cols[:, base + 2:base + 3], in_=sd[:],
                                            axis=AX.X, op=OP.min)
                    # t'' = 255*b + t
                    tpp = pp.tile([128, W], BF16, tag=f"tpp{s}")
                    nc.vector.scalar_tensor_tensor(out=tpp[:], in0=b_t[s][:], scalar=255.0,
                                                   in1=t_t[s][:], op0=OP.mult, op1=OP.add)
                    tpp_t.append(tpp)
                    # CE: load x planes, exp, plane-sum in PSUM
                    r0 = s * 128
                    x = px.tile([128, C * W], BF16, tag="x")
                    for c in range(C):
                        nc.sync.dma_start(x[:, c * W:(c + 1) * W],
                                          xl[img, c, r0:r0 + 128, :])
                    e = px.tile([128, C * W], BF16, tag="e")
                    nc.scalar.activation(e[:], x[:], AF.Exp)
                    se = ps.tile([128, W], F32, tag="se")
                    for c in range(C):
                        nc.tensor.matmul(se[:], Ic, e[:, c * W:(c + 1) * W],
                                         start=(c == 0), stop=(c == C - 1))
                    e_t.append((x, se))

                # ---- P3b: ln + accumulation sums ----
                for s in range(S):
                    j = img * S + s
                    base = j * COLS_PER_STRIP
                    x, se = e_t[s]
                    lse = pt.tile([128, W], F32, tag="lse")
                    nc.scalar.activation(lse[:], se[:], AF.Ln,
                                         accum_out=cols[:, base + 0:base + 1])
                    scr = pt.tile([128, W], F32, tag="scr")
                    nc.vector.scalar_tensor_tensor(
                        out=scr[:], in0=b_t[s][:], scalar=1.0, in1=lse[:],
                        op0=OP.mult, op1=OP.mult,
                        accum_out=cols[:, base + 1:base + 2])
                    scr2 = pt.tile([128, W], BF16, tag="scr2")
                    for c in range(C):
                        nc.vector.scalar_tensor_tensor(
                            out=scr2[:], in0=t_t[s][:], scalar=float(c),
                            in1=x[:, c * W:(c + 1) * W], op0=OP.is_equal, op1=OP.mult,
                            accum_out=cols[:, base + 3 + c:base + 4 + c])
                    for c in range(C):
                        nc.vector.scalar_tensor_tensor(
                            out=scr2[:], in0=tpp_t[s][:], scalar=float(c),
                            in1=x[:, c * W:(c + 1) * W], op0=OP.is_equal, op1=OP.mult,
                            accum_out=cols[:, base + 11 + c:base + 12 + c])

            nc.sync.dma_start(out[:], cols[:])

    _split_sync_waits(nc)
    _NC_CACHE["nc"] = nc
    return nc


def _host_reduce(results):
    """Sum per-core accumulator columns -> loss (f64). Returns (loss, flag_ok)."""
    tot_lse = tot_blse = tot_xt = tot_bxt = 0.0
    flag_ok = True
    for r in results:
        cols = r["out"].astype(np.float64)
        for j in range(NSTRIP):
            base = j * COLS_PER_STRIP
            tot_lse += cols[:, base + 0].sum()
            tot_blse += cols[:, base + 1].sum()
            if cols[:, base + 2].min() <= 0.5:
                flag_ok = False
            tot_xt += cols[:, base + 3:base + 11].sum()
            tot_bxt += cols[:, base + 11:base + 19].sum()
    s1 = tot_lse - tot_xt
    s2 = tot_blse - tot_bxt
    loss = (C1 * s1 + (1.0 - C1) * s2) / NPIX
    return loss, flag_ok


def _pool3(a, op):
    pad = -np.inf if op is np.maximum else np.inf
    p = np.pad(a, ((0, 0), (1, 1), (1, 1)), constant_values=pad)
    r = a.copy()
    for dy in (-1, 0, 1):
        for dx in (-1, 0, 1):
            r = op(r, p[:, 1 + dy:H + 1 + dy, 1 + dx:W + 1 + dx])
    return r


def _fallback(x, t):
    """Exact numpy port of the reference (any input). Slow; never taken for
    generic data -- only when the dilated boundary does not cover an image."""
    tf = t.astype(np.float32)
    bnd = (_pool3(tf, np.maximum) != _pool3(tf, np.minimum)).astype(np.float32)
    dist = np.zeros_like(bnd)
    cur = bnd.copy()
    for i in range(MAX_ITERS):
        dil = _pool3(cur, np.maximum)
        dist += (dil > cur).astype(np.float32) * (i + 1)
        cur = dil
    wts = np.exp(-dist / THETA)
    xm = x.max(axis=1, keepdims=True)
    lse = np.log(np.exp(x - xm).sum(axis=1)) + xm[:, 0]
    xt = np.take_along_axis(x, t[:, None].astype(np.int64), axis=1)[:, 0]
    return np.float32(np.mean((wts * (lse - xt)).astype(np.float64)))


def kernel(inputs, targets):
    x = np.ascontiguousarray(np.asarray(inputs))
    t = np.asarray(targets)
    xb = x.astype(ml_dtypes.bfloat16)
    tb = t.astype(ml_dtypes.bfloat16)
    cst = _band_consts()

    nc = _build_nc()
    in_maps = [
        {"xl": xb[i * PER:(i + 1) * PER], "tg": tb[i * PER:(i + 1) * PER], "cst": cst}
        for i in range(N_CORES)
    ]
    res = run_bass_kernel_spmd(nc, in_maps, list(range(N_CORES)))
    loss, flag_ok = _host_reduce(res.results)
    if not flag_ok:
        return _fallback(x, t)
    return np.float32(loss)


# revision 6
# speedup vs baseline: 27.3388x; 27.3388x over previous
"""BoundaryLoss Trainium2 kernel (8-core data-parallel).

Math (equivalent to the reference for inputs whose boundary dilation covers
the image, which is checked on-device via a flag and falls back otherwise):

  boundary b[p] = 1 iff the 3x3 window around p is NOT single-class
                = (window-sum of adjacent-pixel class-difference indicators) > 0
  If dilate3x3(b) is all-ones, the capped distance transform gives
  weights = c1 + (1-c1)*b  with c1 = exp(-1/theta), so
  loss = [ c1*(Σlse - Σx_t) + (1-c1)*(Σb*lse - Σb*x_t) ] / N
  where lse = logsumexp_c(x), x_t = logit at target class.

Per-core device computation (2 images/core, strips of 128 rows):
  - eh/ev: horizontal/vertical class-difference indicators  (DVE, bf16)
  - window sums: horizontal adds on DVE; vertical via banded-matrix matmuls
    accumulated in PSUM (PE); b = sum > 0
  - dilation flag: 3x3 window-sum of b via PE bands; min-reduce per strip
  - CE: exp on ACT (bf16), plane-sum via identity-matmul PSUM accumulation
    (PE), lse = Ln on ACT with free-dim sum accumulator
  - Σx_t / Σb*x_t: per-class scalar_tensor_tensor (mask-select-multiply with
    row-sum accumulator) on DVE; the b-weighted variant uses a masked target
    t'' = t + 255*b so non-boundary pixels never match a class.
Host sums the per-strip accumulator columns in float64 and applies the
closed form; if any core's flag shows an uncovered pixel, recompute exactly
on host (numpy port of the reference; statistically never taken).
"""
import math
import numpy as np
import ml_dtypes
import concourse.bass as bass
import concourse.tile as tile
from concourse import mybir
from concourse.bass_utils import run_bass_kernel_spmd

BF16 = mybir.dt.bfloat16
F32 = mybir.dt.float32
AF = mybir.ActivationFunctionType
OP = mybir.AluOpType
AX = mybir.AxisListType

B, C, H, W = 16, 8, 512, 512
N_CORES = 8
PER = B // N_CORES            # images per core
S = H // 128                  # strips per image
NSTRIP = PER * S              # strips per core
THETA = 5.0
MAX_ITERS = 15
C1 = math.exp(-1.0 / THETA)
NPIX = B * H * W

# accumulator column layout per strip j: base = j*19
#  +0 lse_sum, +1 blse_sum, +2 flag_min, +3..10 xt[c], +11..18 bxt[c]
COLS_PER_STRIP = 19
NCOLS = NSTRIP * COLS_PER_STRIP


def _split_sync_waits(nc, max_waits=1):
    """Walrus CoreV3 codegen rejects >1 sync wait per instruction; hoist
    extras onto NoOps inserted just before."""
    k = 0
    for f in nc.m.functions:
        for bb in f.blocks:
            new = []
            for ins in bb.instructions:
                w = list(ins.sync_info.on_wait) if ins.sync_info else []
                if len(w) > max_waits:
                    extra, keep = w[:-max_waits], w[-max_waits:]
                    for s0 in range(0, len(extra), max_waits):
                        nop = mybir.InstNoOp(
                            name=f"I-wsplit-{k}", ins=[], outs=[],
                            sync_info=mybir.SyncInfo(
                                on_wait=extra[s0:s0 + max_waits], on_update=[]),
                            engine=ins.engine)
                        k += 1
                        new.append(nop)
                    ins.sync_info.on_wait = keep
                new.append(ins)
            bb.instructions = new


def _band_consts():
    """bf16 [128, 5*128]: T3 (tridiag), T2 (k in {p-1,p}), U (k=127 -> p=0),
    D (k=0 -> p=127), I (identity). lhsT layout: [k, p]."""
    k = np.arange(128)[:, None]
    p = np.arange(128)[None, :]
    T3 = (np.abs(k - p) <= 1).astype(np.float32)
    T2 = ((k == p) | (k == p - 1)).astype(np.float32)
    U = ((k == 127) & (p == 0)).astype(np.float32)
    D = ((k == 0) & (p == 127)).astype(np.float32)
    I = (k == p).astype(np.float32)
    return np.concatenate([T3, T2, U, D, I], axis=1).astype(ml_dtypes.bfloat16)


_NC_CACHE = {}


def _build_nc(repeat=1):
    """repeat>1 re-runs the whole per-core computation, overwriting the same
    accumulators -- output equals the repeat=1 result; used for timing."""
    if repeat in _NC_CACHE:
        return _NC_CACHE[repeat]
    nc = bass.Bass()
    xl = nc.dram_tensor("xl", [PER, C, H, W], BF16, kind="ExternalInput")
    tg = nc.dram_tensor("tg", [PER, H, W], BF16, kind="ExternalInput")
    cst = nc.dram_tensor("cst", [128, 5 * 128], BF16, kind="ExternalInput")
    out = nc.dram_tensor("out", [128, NCOLS], F32, kind="ExternalOutput")

    with tile.TileContext(nc) as tc:
        with (
            tc.tile_pool(name="pc", bufs=1) as pc,
            tc.tile_pool(name="pp", bufs=2) as pp,      # persistent per-strip maps
            tc.tile_pool(name="pt", bufs=3) as pt,      # transients
            tc.tile_pool(name="px", bufs=3) as px,      # big x tiles
            tc.tile_pool(name="pa", bufs=1) as pa,      # accumulator columns
            tc.tile_pool(name="ps", bufs=2, space="PSUM") as ps,
        ):
            cons = pc.tile([128, 5 * 128], BF16, tag="cons")
            nc.sync.dma_start(cons[:], cst[:])
            T3 = cons[:, 0:128]
            T2 = cons[:, 128:256]
            Uc = cons[:, 256:384]
            Dc = cons[:, 384:512]
            Ic = cons[:, 512:640]

            cols = pa.tile([128, NCOLS], F32, tag="cols")

            for img in [i % PER for i in range(repeat * PER)]:
                t_t, h2_t, h3_t, b_t, h3b_t = [], [], [], [], []
                # ---- P1: load t, edge maps, horizontal window sums ----
                for s in range(S):
                    r0 = s * 128
                    t = pp.tile([128, W], BF16, tag=f"t{s}")
                    nc.gpsimd.dma_start(t[:], tg[img, r0:r0 + 128, :])
                    td = pt.tile([128, W], BF16, tag="td")
                    if s < S - 1:
                        nc.gpsimd.dma_start(td[:], tg[img, r0 + 1:r0 + 129, :])
                    else:
                        nc.gpsimd.dma_start(td[0:127, :], tg[img, r0 + 1:r0 + 128, :])
                        nc.gpsimd.dma_start(td[127:128, :], tg[img, H - 1:H, :])
                    # eh[c] = t[c] != t[c+1], c<511; col 511 = 0
                    eh = pt.tile([128, W], BF16, tag="eh")
                    nc.gpsimd.memset(eh[:, W - 1:W], 0.0)
                    nc.vector.tensor_tensor(out=eh[:, 0:W - 1], in0=t[:, 0:W - 1],
                                            in1=t[:, 1:W], op=OP.not_equal)
                    # ev = t != t_down   (row 511 clamps to itself -> 0)
                    ev = pt.tile([128, W], BF16, tag="ev")
                    nc.vector.tensor_tensor(out=ev[:], in0=t[:], in1=td[:],
                                            op=OP.not_equal)
                    # H2eh[c] = eh[c-1] + eh[c]
                    h2 = pp.tile([128, W], BF16, tag=f"h2{s}")
                    nc.gpsimd.tensor_copy(h2[:, 0:1], eh[:, 0:1])
                    nc.vector.tensor_tensor(out=h2[:, 1:W], in0=eh[:, 0:W - 1],
                                            in1=eh[:, 1:W], op=OP.add)
                    # H3ev[c] = ev[c-1] + ev[c] + ev[c+1]
                    h3 = pp.tile([128, W], BF16, tag=f"h3{s}")
                    tmp = pt.tile([128, W], BF16, tag="tmp")
                    nc.vector.tensor_tensor(out=tmp[:, 0:W - 1], in0=ev[:, 0:W - 1],
                                            in1=ev[:, 1:W], op=OP.add)
                    nc.vector.tensor_tensor(out=h3[:, 1:W - 1], in0=tmp[:, 0:W - 2],
                                            in1=ev[:, 2:W], op=OP.add)
                    nc.gpsimd.tensor_copy(h3[:, 0:1], tmp[:, 0:1])
                    nc.gpsimd.tensor_copy(h3[:, W - 1:W], tmp[:, W - 2:W - 1])
                    t_t.append(t); h2_t.append(h2); h3_t.append(h3)

                # ---- P2: vertical sums via PE bands -> b; then H3 of b ----
                for s in range(S):
                    sb = ps.tile([128, W], F32, tag="sb")
                    nc.tensor.matmul(sb[:], T3, h2_t[s][:], start=True, stop=False)
                    if s > 0:
                        nc.tensor.matmul(sb[:], Uc, h2_t[s - 1][:], start=False, stop=False)
                    if s < S - 1:
                        nc.tensor.matmul(sb[:], Dc, h2_t[s + 1][:], start=False, stop=False)
                    nc.tensor.matmul(sb[:], T2, h3_t[s][:], start=False, stop=(s == 0))
                    if s > 0:
                        nc.tensor.matmul(sb[:], Uc, h3_t[s - 1][:], start=False, stop=True)
                    b = pp.tile([128, W], BF16, tag=f"b{s}")
                    nc.vector.tensor_scalar(out=b[:], in0=sb[:], scalar1=0.5,
                                            scalar2=None, op0=OP.is_gt)
                    # H3b
                    h3b = pp.tile([128, W], BF16, tag=f"h3b{s}")
                    tmp2 = pt.tile([128, W], BF16, tag="tmp2")
                    nc.vector.tensor_tensor(out=tmp2[:, 0:W - 1], in0=b[:, 0:W - 1],
                                            in1=b[:, 1:W], op=OP.add)
                    nc.vector.tensor_tensor(out=h3b[:, 1:W - 1], in0=tmp2[:, 0:W - 2],
                                            in1=b[:, 2:W], op=OP.add)
                    nc.gpsimd.tensor_copy(h3b[:, 0:1], tmp2[:, 0:1])
                    nc.gpsimd.tensor_copy(h3b[:, W - 1:W], tmp2[:, W - 2:W - 1])
                    b_t.append(b); h3b_t.append(h3b)

                # ---- P3a: dilation flag + t'' + exp + plane sums ----
                e_t, lse_t, tpp_t = [], [], []
                for s in range(S):
                    j = img * S + s
                    base = j * COLS_PER_STRIP
                    sd = ps.tile([128, W], F32, tag="sd")
                    nc.tensor.matmul(sd[:], T3, h3b_t[s][:], start=True,
                                     stop=(s == 0 and S == 1))
                    if s > 0:
                        nc.tensor.matmul(sd[:], Uc, h3b_t[s - 1][:], start=False,
                                         stop=(s == S - 1))
                    if s < S - 1:
                        nc.tensor.matmul(sd[:], Dc, h3b_t[s + 1][:], start=False,
                                         stop=True)
                    # min over dilation sums; host checks > 0
                    nc.vector.tensor_reduce(out=cols[:, base + 2:base + 3], in_=sd[:],
                                            axis=AX.X, op=OP.min)
                    # t'' = 255*b + t
                    tpp = pp.tile([128, W], BF16, tag=f"tpp{s}")
                    nc.vector.scalar_tensor_tensor(out=tpp[:], in0=b_t[s][:], scalar=255.0,
                                                   in1=t_t[s][:], op0=OP.mult, op1=OP.add)
                    tpp_t.append(tpp)
                    # CE: load x planes, exp, plane-sum in PSUM
                    r0 = s * 128
                    x = px.tile([128, C * W], BF16, tag="x")
                    for c in range(C):
                        nc.sync.dma_start(x[:, c * W:(c + 1) * W],
                                          xl[img, c, r0:r0 + 128, :])
                    e = px.tile([128, C * W], BF16, tag="e")
                    nc.scalar.activation(e[:], x[:], AF.Exp)
                    se = ps.tile([128, W], F32, tag="se")
                    for c in range(C):
                        nc.tensor.matmul(se[:], Ic, e[:, c * W:(c + 1) * W],
                                         start=(c == 0), stop=(c == C - 1))
                    e_t.append((x, se))

                # ---- P3b: ln + accumulation sums ----
                for s in range(S):
                    j = img * S + s
                    base = j * COLS_PER_STRIP
                    x, se = e_t[s]
                    lse = pt.tile([128, W], F32, tag="lse")
                    nc.scalar.activation(lse[:], se[:], AF.Ln,
                                         accum_out=cols[:, base + 0:base + 1])
                    scr = pt.tile([128, W], F32, tag="scr")
                    nc.vector.scalar_tensor_tensor(
                        out=scr[:], in0=b_t[s][:], scalar=1.0, in1=lse[:],
                        op0=OP.mult, op1=OP.mult,
                        accum_out=cols[:, base + 1:base + 2])
                    scr2 = pt.tile([128, W], BF16, tag="scr2")
                    for c in range(C):
                        nc.vector.scalar_tensor_tensor(
                            out=scr2[:], in0=t_t[s][:], scalar=float(c),
                            in1=x[:, c * W:(c + 1) * W], op0=OP.is_equal, op1=OP.mult,
                            accum_out=cols[:, base + 3 + c:base + 4 + c])
                    for c in range(C):
                        nc.vector.scalar_tensor_tensor(
                            out=scr2[:], in0=tpp_t[s][:], scalar=float(c),
                            in1=x[:, c * W:(c + 1) * W], op0=OP.is_equal, op1=OP.mult,
                            accum_out=cols[:, base + 11 + c:base + 12 + c])

            nc.sync.dma_start(out[:], cols[:])

    _split_sync_waits(nc)
    _NC_CACHE[repeat] = nc
    return nc


def _host_reduce(results):
    """Sum per-core accumulator columns -> loss (f64). Returns (loss, flag_ok)."""
    tot_lse = tot_blse = tot_xt = tot_bxt = 0.0
    flag_ok = True
    for r in results:
        cols = r["out"].astype(np.float64)
        for j in range(NSTRIP):
            base = j * COLS_PER_STRIP
            tot_lse += cols[:, base + 0].sum()
            tot_blse += cols[:, base + 1].sum()
            if cols[:, base + 2].min() <= 0.5:
                flag_ok = False
            tot_xt += cols[:, base + 3:base + 11].sum()
            tot_bxt += cols[:, base + 11:base + 19].sum()
    s1 = tot_lse - tot_xt
    s2 = tot_blse - tot_bxt
    loss = (C1 * s1 + (1.0 - C1) * s2) / NPIX
    return loss, flag_ok


def _pool3(a, op):
    pad = -np.inf if op is np.maximum else np.inf
    p = np.pad(a, ((0, 0), (1, 1), (1, 1)), constant_values=pad)
    r = a.copy()
    for dy in (-1, 0, 1):
        for dx in (-1, 0, 1):
            r = op(r, p[:, 1 + dy:H + 1 + dy, 1 + dx:W + 1 + dx])
    return r


def _fallback(x, t):
    """Exact numpy port of the reference (any input). Slow; never taken for
    generic data -- only when the dilated boundary does not cover an image."""
    tf = t.astype(np.float32)
    bnd = (_pool3(tf, np.maximum) != _pool3(tf, np.minimum)).astype(np.float32)
    dist = np.zeros_like(bnd)
    cur = bnd.copy()
    for i in range(MAX_ITERS):
        dil = _pool3(cur, np.maximum)
        dist += (dil > cur).astype(np.float32) * (i + 1)
        cur = dil
    wts = np.exp(-dist / THETA)
    xm = x.max(axis=1, keepdims=True)
    lse = np.log(np.exp(x - xm).sum(axis=1)) + xm[:, 0]
    xt = np.take_along_axis(x, t[:, None].astype(np.int64), axis=1)[:, 0]
    return np.float32(np.mean((wts * (lse - xt)).astype(np.float64)))


def kernel(inputs, targets):
    x = np.ascontiguousarray(np.asarray(inputs))
    t = np.asarray(targets)
    xb = x.astype(ml_dtypes.bfloat16)
    tb = t.astype(ml_dtypes.bfloat16)
    cst = _band_consts()

    nc = _build_nc()
    in_maps = [
        {"xl": xb[i * PER:(i + 1) * PER], "tg": tb[i * PER:(i + 1) * PER], "cst": cst}
        for i in range(N_CORES)
    ]
    res = run_bass_kernel_spmd(nc, in_maps, list(range(N_CORES)))
    loss, flag_ok = _host_reduce(res.results)
    if not flag_ok:
        return _fallback(x, t)
    return np.float32(loss)


# revision 7
# speedup vs baseline: 5912.8286x; 216.2794x over previous
"""BoundaryLoss Trainium2 kernel (8-core data-parallel).

Math (equivalent to the reference for inputs whose boundary dilation covers
the image, which is checked on-device via a flag and falls back otherwise):

  boundary b[p] = 1 iff the 3x3 window around p is NOT single-class
                = (window-sum of adjacent-pixel class-difference indicators) > 0
  If dilate3x3(b) is all-ones, the capped distance transform gives
  weights = c1 + (1-c1)*b  with c1 = exp(-1/theta), so
  loss = [ c1*(Σlse - Σx_t) + (1-c1)*(Σb*lse - Σb*x_t) ] / N
  where lse = logsumexp_c(x), x_t = logit at target class.

Per-core device computation (2 images/core, strips of 128 rows):
  - eh/ev: horizontal/vertical class-difference indicators  (DVE, bf16)
  - window sums: horizontal adds on DVE; vertical via banded-matrix matmuls
    accumulated in PSUM (PE); b = sum > 0
  - dilation flag: 3x3 window-sum of b via PE bands; min-reduce per strip
  - CE: exp on ACT (bf16), plane-sum via identity-matmul PSUM accumulation
    (PE), lse = Ln on ACT with free-dim sum accumulator
  - Σx_t / Σb*x_t: per-class scalar_tensor_tensor (mask-select-multiply with
    row-sum accumulator) on DVE; the b-weighted variant uses a masked target
    t'' = t + 255*b so non-boundary pixels never match a class.
Host sums the per-strip accumulator columns in float64 and applies the
closed form; if any core's flag shows an uncovered pixel, recompute exactly
on host (numpy port of the reference; statistically never taken).
"""
import math
import numpy as np
import ml_dtypes
import concourse.bass as bass
import concourse.tile as tile
from concourse import mybir
from concourse.bass_utils import run_bass_kernel_spmd

BF16 = mybir.dt.bfloat16
F32 = mybir.dt.float32
AF = mybir.ActivationFunctionType
OP = mybir.AluOpType
AX = mybir.AxisListType

B, C, H, W = 16, 8, 512, 512
N_CORES = 8
PER = B // N_CORES            # images per core
S = H // 128                  # strips per image
NSTRIP = PER * S              # strips per core
THETA = 5.0
MAX_ITERS = 15
C1 = math.exp(-1.0 / THETA)
NPIX = B * H * W

# accumulator column layout per strip j: base = j*19
#  +0 lse_sum, +1 blse_sum, +2 flag_min, +3..10 xt[c], +11..18 bxt[c]
COLS_PER_STRIP = 19
NCOLS = NSTRIP * COLS_PER_STRIP


def _split_sync_waits(nc, max_waits=1):
    """Walrus CoreV3 codegen rejects >1 sync wait per instruction; hoist
    extras onto NoOps inserted just before."""
    k = 0
    for f in nc.m.functions:
        for bb in f.blocks:
            new = []
            for ins in bb.instructions:
                w = list(ins.sync_info.on_wait) if ins.sync_info else []
                if len(w) > max_waits:
                    extra, keep = w[:-max_waits], w[-max_waits:]
                    for s0 in range(0, len(extra), max_waits):
                        nop = mybir.InstNoOp(
                            name=f"I-wsplit-{k}", ins=[], outs=[],
                            sync_info=mybir.SyncInfo(
                                on_wait=extra[s0:s0 + max_waits], on_update=[]),
                            engine=ins.engine)
                        k += 1
                        new.append(nop)
                    ins.sync_info.on_wait = keep
                new.append(ins)
            bb.instructions = new


def _band_consts():
    """bf16 [128, 5*128]: T3 (tridiag), T2 (k in {p-1,p}), U (k=127 -> p=0),
    D (k=0 -> p=127), I (identity). lhsT layout: [k, p]."""
    k = np.arange(128)[:, None]
    p = np.arange(128)[None, :]
    T3 = (np.abs(k - p) <= 1).astype(np.float32)
    T2 = ((k == p) | (k == p - 1)).astype(np.float32)
    U = ((k == 127) & (p == 0)).astype(np.float32)
    D = ((k == 0) & (p == 127)).astype(np.float32)
    I = (k == p).astype(np.float32)
    return np.concatenate([T3, T2, U, D, I], axis=1).astype(ml_dtypes.bfloat16)


_NC_CACHE = {}


def _build_nc(repeat=1, split=True):
    """repeat>1 re-runs the whole per-core computation, overwriting the same
    accumulators -- output equals the repeat=1 result; used for timing."""
    key = (repeat, split)
    if key in _NC_CACHE:
        return _NC_CACHE[key]
    nc = bass.Bass()
    xl = nc.dram_tensor("xl", [PER, C, H, W], BF16, kind="ExternalInput")
    tg = nc.dram_tensor("tg", [PER, H, W], BF16, kind="ExternalInput")
    cst = nc.dram_tensor("cst", [128, 5 * 128], BF16, kind="ExternalInput")
    out = nc.dram_tensor("out", [128, NCOLS], F32, kind="ExternalOutput")

    with tile.TileContext(nc) as tc:
        with (
            tc.tile_pool(name="pc", bufs=1) as pc,
            tc.tile_pool(name="pp", bufs=2) as pp,      # persistent per-strip maps
            tc.tile_pool(name="pt", bufs=3) as pt,      # transients
            tc.tile_pool(name="px", bufs=3) as px,      # big x tiles
            tc.tile_pool(name="pa", bufs=1) as pa,      # accumulator columns
            tc.tile_pool(name="ps", bufs=2, space="PSUM") as ps,
        ):
            cons = pc.tile([128, 5 * 128], BF16, tag="cons")
            nc.sync.dma_start(cons[:], cst[:])
            T3 = cons[:, 0:128]
            T2 = cons[:, 128:256]
            Uc = cons[:, 256:384]
            Dc = cons[:, 384:512]
            Ic = cons[:, 512:640]

            cols = pa.tile([128, NCOLS], F32, tag="cols")

            for img in [i % PER for i in range(repeat * PER)]:
                t_t, h2_t, h3_t, b_t, h3b_t = [], [], [], [], []
                # ---- P1: load t, edge maps, horizontal window sums ----
                for s in range(S):
                    r0 = s * 128
                    t = pp.tile([128, W], BF16, tag=f"t{s}")
                    nc.gpsimd.dma_start(t[:], tg[img, r0:r0 + 128, :])
                    td = pt.tile([128, W], BF16, tag="td")
                    if s < S - 1:
                        nc.gpsimd.dma_start(td[:], tg[img, r0 + 1:r0 + 129, :])
                    else:
                        nc.gpsimd.dma_start(td[0:127, :], tg[img, r0 + 1:r0 + 128, :])
                        nc.gpsimd.dma_start(td[127:128, :], tg[img, H - 1:H, :])
                    # eh[c] = t[c] != t[c+1], c<511; col 511 = 0
                    eh = pt.tile([128, W], BF16, tag="eh")
                    nc.gpsimd.memset(eh[:, W - 1:W], 0.0)
                    nc.vector.tensor_tensor(out=eh[:, 0:W - 1], in0=t[:, 0:W - 1],
                                            in1=t[:, 1:W], op=OP.not_equal)
                    # ev = t != t_down   (row 511 clamps to itself -> 0)
                    ev = pt.tile([128, W], BF16, tag="ev")
                    nc.vector.tensor_tensor(out=ev[:], in0=t[:], in1=td[:],
                                            op=OP.not_equal)
                    # H2eh[c] = eh[c-1] + eh[c]
                    h2 = pp.tile([128, W], BF16, tag=f"h2{s}")
                    nc.gpsimd.tensor_copy(h2[:, 0:1], eh[:, 0:1])
                    nc.vector.tensor_tensor(out=h2[:, 1:W], in0=eh[:, 0:W - 1],
                                            in1=eh[:, 1:W], op=OP.add)
                    # H3ev[c] = ev[c-1] + ev[c] + ev[c+1]
                    h3 = pp.tile([128, W], BF16, tag=f"h3{s}")
                    tmp = pt.tile([128, W], BF16, tag="tmp")
                    nc.vector.tensor_tensor(out=tmp[:, 0:W - 1], in0=ev[:, 0:W - 1],
                                            in1=ev[:, 1:W], op=OP.add)
                    nc.vector.tensor_tensor(out=h3[:, 1:W - 1], in0=tmp[:, 0:W - 2],
                                            in1=ev[:, 2:W], op=OP.add)
                    nc.gpsimd.tensor_copy(h3[:, 0:1], tmp[:, 0:1])
                    nc.gpsimd.tensor_copy(h3[:, W - 1:W], tmp[:, W - 2:W - 1])
                    t_t.append(t); h2_t.append(h2); h3_t.append(h3)

                # ---- P2: vertical sums via PE bands -> b; then H3 of b ----
                for s in range(S):
                    sb = ps.tile([128, W], F32, tag="sb")
                    nc.tensor.matmul(sb[:], T3, h2_t[s][:], start=True, stop=False)
                    if s > 0:
                        nc.tensor.matmul(sb[:], Uc, h2_t[s - 1][:], start=False, stop=False)
                    if s < S - 1:
                        nc.tensor.matmul(sb[:], Dc, h2_t[s + 1][:], start=False, stop=False)
                    nc.tensor.matmul(sb[:], T2, h3_t[s][:], start=False, stop=(s == 0))
                    if s > 0:
                        nc.tensor.matmul(sb[:], Uc, h3_t[s - 1][:], start=False, stop=True)
                    b = pp.tile([128, W], BF16, tag=f"b{s}")
                    nc.vector.tensor_scalar(out=b[:], in0=sb[:], scalar1=0.5,
                                            scalar2=None, op0=OP.is_gt)
                    # H3b
                    h3b = pp.tile([128, W], BF16, tag=f"h3b{s}")
                    tmp2 = pt.tile([128, W], BF16, tag="tmp2")
                    nc.vector.tensor_tensor(out=tmp2[:, 0:W - 1], in0=b[:, 0:W - 1],
                                            in1=b[:, 1:W], op=OP.add)
                    nc.vector.tensor_tensor(out=h3b[:, 1:W - 1], in0=tmp2[:, 0:W - 2],
                                            in1=b[:, 2:W], op=OP.add)
                    nc.gpsimd.tensor_copy(h3b[:, 0:1], tmp2[:, 0:1])
                    nc.gpsimd.tensor_copy(h3b[:, W - 1:W], tmp2[:, W - 2:W - 1])
                    b_t.append(b); h3b_t.append(h3b)

                # ---- P3a: dilation flag + t'' + exp + plane sums ----
                e_t, lse_t, tpp_t = [], [], []
                for s in range(S):
                    j = img * S + s
                    base = j * COLS_PER_STRIP
                    sd = ps.tile([128, W], F32, tag="sd")
                    nc.tensor.matmul(sd[:], T3, h3b_t[s][:], start=True,
                                     stop=(s == 0 and S == 1))
                    if s > 0:
                        nc.tensor.matmul(sd[:], Uc, h3b_t[s - 1][:], start=False,
                                         stop=(s == S - 1))
                    if s < S - 1:
                        nc.tensor.matmul(sd[:], Dc, h3b_t[s + 1][:], start=False,
                                         stop=True)
                    # min over dilation sums; host checks > 0
                    nc.vector.tensor_reduce(out=cols[:, base + 2:base + 3], in_=sd[:],
                                            axis=AX.X, op=OP.min)
                    # t'' = 255*b + t
                    tpp = pp.tile([128, W], BF16, tag=f"tpp{s}")
                    nc.vector.scalar_tensor_tensor(out=tpp[:], in0=b_t[s][:], scalar=255.0,
                                                   in1=t_t[s][:], op0=OP.mult, op1=OP.add)
                    tpp_t.append(tpp)
                    # CE: load x planes, exp, plane-sum in PSUM
                    r0 = s * 128
                    x = px.tile([128, C * W], BF16, tag="x")
                    for c in range(C):
                        nc.sync.dma_start(x[:, c * W:(c + 1) * W],
                                          xl[img, c, r0:r0 + 128, :])
                    e = px.tile([128, C * W], BF16, tag="e")
                    nc.scalar.activation(e[:], x[:], AF.Exp)
                    se = ps.tile([128, W], F32, tag="se")
                    for c in range(C):
                        nc.tensor.matmul(se[:], Ic, e[:, c * W:(c + 1) * W],
                                         start=(c == 0), stop=(c == C - 1))
                    e_t.append((x, se))

                # ---- P3b: ln + accumulation sums ----
                for s in range(S):
                    j = img * S + s
                    base = j * COLS_PER_STRIP
                    x, se = e_t[s]
                    lse = pt.tile([128, W], F32, tag="lse")
                    nc.scalar.activation(lse[:], se[:], AF.Ln,
                                         accum_out=cols[:, base + 0:base + 1])
                    scr = pt.tile([128, W], F32, tag="scr")
                    nc.vector.scalar_tensor_tensor(
                        out=scr[:], in0=b_t[s][:], scalar=1.0, in1=lse[:],
                        op0=OP.mult, op1=OP.mult,
                        accum_out=cols[:, base + 1:base + 2])
                    scr2 = pt.tile([128, W], BF16, tag="scr2")
                    for c in range(C):
                        nc.vector.scalar_tensor_tensor(
                            out=scr2[:], in0=t_t[s][:], scalar=float(c),
                            in1=x[:, c * W:(c + 1) * W], op0=OP.is_equal, op1=OP.mult,
                            accum_out=cols[:, base + 3 + c:base + 4 + c])
                    for c in range(C):
                        nc.vector.scalar_tensor_tensor(
                            out=scr2[:], in0=tpp_t[s][:], scalar=float(c),
                            in1=x[:, c * W:(c + 1) * W], op0=OP.is_equal, op1=OP.mult,
                            accum_out=cols[:, base + 11 + c:base + 12 + c])

            nc.sync.dma_start(out[:], cols[:])

    if split:
        _split_sync_waits(nc)
    _NC_CACHE[key] = nc
    return nc


def _host_reduce(results):
    """Sum per-core accumulator columns -> loss (f64). Returns (loss, flag_ok)."""
    tot_lse = tot_blse = tot_xt = tot_bxt = 0.0
    flag_ok = True
    for r in results:
        cols = r["out"].astype(np.float64)
        for j in range(NSTRIP):
            base = j * COLS_PER_STRIP
            tot_lse += cols[:, base + 0].sum()
            tot_blse += cols[:, base + 1].sum()
            if cols[:, base + 2].min() <= 0.5:
                flag_ok = False
            tot_xt += cols[:, base + 3:base + 11].sum()
            tot_bxt += cols[:, base + 11:base + 19].sum()
    s1 = tot_lse - tot_xt
    s2 = tot_blse - tot_bxt
    loss = (C1 * s1 + (1.0 - C1) * s2) / NPIX
    return loss, flag_ok


def _pool3(a, op):
    pad = -np.inf if op is np.maximum else np.inf
    p = np.pad(a, ((0, 0), (1, 1), (1, 1)), constant_values=pad)
    r = a.copy()
    for dy in (-1, 0, 1):
        for dx in (-1, 0, 1):
            r = op(r, p[:, 1 + dy:H + 1 + dy, 1 + dx:W + 1 + dx])
    return r


def _fallback(x, t):
    """Exact numpy port of the reference (any input). Slow; never taken for
    generic data -- only when the dilated boundary does not cover an image."""
    tf = t.astype(np.float32)
    bnd = (_pool3(tf, np.maximum) != _pool3(tf, np.minimum)).astype(np.float32)
    dist = np.zeros_like(bnd)
    cur = bnd.copy()
    for i in range(MAX_ITERS):
        dil = _pool3(cur, np.maximum)
        dist += (dil > cur).astype(np.float32) * (i + 1)
        cur = dil
    wts = np.exp(-dist / THETA)
    xm = x.max(axis=1, keepdims=True)
    lse = np.log(np.exp(x - xm).sum(axis=1)) + xm[:, 0]
    xt = np.take_along_axis(x, t[:, None].astype(np.int64), axis=1)[:, 0]
    return np.float32(np.mean((wts * (lse - xt)).astype(np.float64)))


def kernel(inputs, targets):
    x = np.ascontiguousarray(np.asarray(inputs))
    t = np.asarray(targets)
    xb = x.astype(ml_dtypes.bfloat16)
    tb = t.astype(ml_dtypes.bfloat16)
    cst = _band_consts()

    nc = _build_nc()
    in_maps = [
        {"xl": xb[i * PER:(i + 1) * PER], "tg": tb[i * PER:(i + 1) * PER], "cst": cst}
        for i in range(N_CORES)
    ]
    res = run_bass_kernel_spmd(nc, in_maps, list(range(N_CORES)))
    loss, flag_ok = _host_reduce(res.results)
    if not flag_ok:
        return _fallback(x, t)
    return np.float32(loss)


# revision 8
# speedup vs baseline: 18088.5398x; 3.0592x over previous
"""BoundaryLoss Trainium2 kernel (8-core data-parallel).

Math: boundary b[p] = 1 iff the 3x3 window around p spans >1 class.  The
reference's capped iterative distance transform assigns dist=0 to boundary
pixels, dist=D (chebyshev distance to the boundary) for 1<=D<=15, dist=0
beyond.  A pixel with D>=2 requires a fully non-boundary 3x3 block, i.e. at
least 9 non-boundary pixels in the image set; when the total non-boundary
count is < 9 (always, for random multi-class targets), every non-boundary
pixel has D==1 and the weights collapse to  w = c1 + (1-c1)*b,
c1 = exp(-1/theta).  Then

  loss * N = sum(ce) - (1-c1) * sum_{b==0}(ce),   ce = lse - x_t

The second term touches <9 pixels; the host computes it exactly in f64 from
the device-produced boundary map.  If the screen fails (>=9 non-boundary
pixels) the host falls back to an exact numpy port of the reference.

Per-core device computation (2 images/core, strips of 128 rows):
  - eh/ev: horizontal/vertical class-difference indicators (DVE, bf16, 2x)
  - 3x3 window sum of the indicators: horizontal adds on DVE; vertical via
    banded-matrix matmuls accumulated in PSUM (PE); b = sum > 0 -> uint8 map
  - CE: exp on ACT (bf16), class-plane sum via identity-matmul PSUM
    accumulation (PE), lse = Ln on ACT with free-dim sum accumulator
  - sum(x_t): per-class scalar_tensor_tensor (t==c)*x_c with row-sum
    accumulator on DVE
Host: sums accumulator columns in f64, applies the sparse correction.
"""
import math
import numpy as np
import ml_dtypes
import concourse.bass as bass
import concourse.tile as tile
from concourse import mybir
from concourse.bass_utils import run_bass_kernel_spmd

BF16 = mybir.dt.bfloat16
F32 = mybir.dt.float32
U8 = mybir.dt.uint8
AF = mybir.ActivationFunctionType
OP = mybir.AluOpType
AX = mybir.AxisListType

B, C, H, W = 16, 8, 512, 512
N_CORES = 8
PER = B // N_CORES            # images per core
S = H // 128                  # strips per image
NSTRIP = PER * S              # strips per core
THETA = 5.0
MAX_ITERS = 15
C1 = math.exp(-1.0 / THETA)
NPIX = B * H * W

# accumulator column layout per strip j: base = j*9: +0 lse_sum, +1..8 xt[c]
COLS_PER_STRIP = 9
NCOLS = NSTRIP * COLS_PER_STRIP


def _split_sync_waits(nc, max_waits=1):
    """Walrus CoreV3 codegen rejects >1 sync wait per instruction; hoist
    extras onto NoOps inserted just before."""
    k = 0
    for f in nc.m.functions:
        for bb in f.blocks:
            new = []
            for ins in bb.instructions:
                w = list(ins.sync_info.on_wait) if ins.sync_info else []
                if len(w) > max_waits:
                    extra, keep = w[:-max_waits], w[-max_waits:]
                    for s0 in range(0, len(extra), max_waits):
                        nop = mybir.InstNoOp(
                            name=f"I-wsplit-{k}", ins=[], outs=[],
                            sync_info=mybir.SyncInfo(
                                on_wait=extra[s0:s0 + max_waits], on_update=[]),
                            engine=ins.engine)
                        k += 1
                        new.append(nop)
                    ins.sync_info.on_wait = keep
                new.append(ins)
            bb.instructions = new


def _band_consts():
    """bf16 [128, 5*128]: T3 (tridiag), T2 (k in {p-1,p}), U (k=127 -> p=0),
    D (k=0 -> p=127), I (identity). lhsT layout: [k, p]."""
    k = np.arange(128)[:, None]
    p = np.arange(128)[None, :]
    T3 = (np.abs(k - p) <= 1).astype(np.float32)
    T2 = ((k == p) | (k == p - 1)).astype(np.float32)
    U = ((k == 127) & (p == 0)).astype(np.float32)
    D = ((k == 0) & (p == 127)).astype(np.float32)
    I = (k == p).astype(np.float32)
    return np.concatenate([T3, T2, U, D, I], axis=1).astype(ml_dtypes.bfloat16)


_NC_CACHE = {}


def _build_nc(repeat=1, split=True):
    """repeat>1 re-runs the whole per-core computation, overwriting the same
    accumulators -- output equals the repeat=1 result; used for timing."""
    key = (repeat, split)
    if key in _NC_CACHE:
        return _NC_CACHE[key]
    nc = bass.Bass()
    xl = nc.dram_tensor("xl", [PER, C, H, W], BF16, kind="ExternalInput")
    tg = nc.dram_tensor("tg", [PER, H, W], BF16, kind="ExternalInput")
    cst = nc.dram_tensor("cst", [128, 5 * 128], BF16, kind="ExternalInput")
    out = nc.dram_tensor("out", [128, NCOLS], F32, kind="ExternalOutput")
    bm = nc.dram_tensor("bm", [PER, H, W], U8, kind="ExternalOutput")

    with tile.TileContext(nc) as tc:
        with (
            tc.tile_pool(name="pc", bufs=1) as pc,
            tc.tile_pool(name="pp", bufs=2) as pp,      # persistent per-strip maps
            tc.tile_pool(name="pt", bufs=3) as pt,      # transients
            tc.tile_pool(name="px", bufs=3) as px,      # big x tiles
            tc.tile_pool(name="pa", bufs=1) as pa,      # accumulator columns
            tc.tile_pool(name="ps", bufs=2, space="PSUM") as ps,
        ):
            cons = pc.tile([128, 5 * 128], BF16, tag="cons")
            nc.sync.dma_start(cons[:], cst[:])
            T3 = cons[:, 0:128]
            T2 = cons[:, 128:256]
            Uc = cons[:, 256:384]
            Dc = cons[:, 384:512]
            Ic = cons[:, 512:640]

            cols = pa.tile([128, NCOLS], F32, tag="cols")

            for img in [i % PER for i in range(repeat * PER)]:
                t_t, h2_t, h3_t = [], [], []
                # ---- P1: load t, edge maps, horizontal window sums ----
                for s in range(S):
                    r0 = s * 128
                    t = pp.tile([128, W], BF16, tag=f"t{s}")
                    nc.gpsimd.dma_start(t[:], tg[img, r0:r0 + 128, :])
                    td = pt.tile([128, W], BF16, tag="td")
                    if s < S - 1:
                        nc.gpsimd.dma_start(td[:], tg[img, r0 + 1:r0 + 129, :])
                    else:
                        nc.gpsimd.dma_start(td[0:127, :], tg[img, r0 + 1:r0 + 128, :])
                        nc.gpsimd.dma_start(td[127:128, :], tg[img, H - 1:H, :])
                    # eh[c] = t[c] != t[c+1], c<511; col 511 = 0
                    eh = pt.tile([128, W], BF16, tag="eh")
                    nc.gpsimd.memset(eh[:, W - 1:W], 0.0)
                    nc.vector.tensor_tensor(out=eh[:, 0:W - 1], in0=t[:, 0:W - 1],
                                            in1=t[:, 1:W], op=OP.not_equal)
                    # ev = t != t_down   (row 511 clamps to itself -> 0)
                    ev = pt.tile([128, W], BF16, tag="ev")
                    nc.vector.tensor_tensor(out=ev[:], in0=t[:], in1=td[:],
                                            op=OP.not_equal)
                    # H2eh[c] = eh[c-1] + eh[c]
                    h2 = pp.tile([128, W], BF16, tag=f"h2{s}")
                    nc.gpsimd.tensor_copy(h2[:, 0:1], eh[:, 0:1])
                    nc.vector.tensor_tensor(out=h2[:, 1:W], in0=eh[:, 0:W - 1],
                                            in1=eh[:, 1:W], op=OP.add)
                    # H3ev[c] = ev[c-1] + ev[c] + ev[c+1]
                    h3 = pp.tile([128, W], BF16, tag=f"h3{s}")
                    tmp = pt.tile([128, W], BF16, tag="tmp")
                    nc.vector.tensor_tensor(out=tmp[:, 0:W - 1], in0=ev[:, 0:W - 1],
                                            in1=ev[:, 1:W], op=OP.add)
                    nc.vector.tensor_tensor(out=h3[:, 1:W - 1], in0=tmp[:, 0:W - 2],
                                            in1=ev[:, 2:W], op=OP.add)
                    nc.gpsimd.tensor_copy(h3[:, 0:1], tmp[:, 0:1])
                    nc.gpsimd.tensor_copy(h3[:, W - 1:W], tmp[:, W - 2:W - 1])
                    t_t.append(t); h2_t.append(h2); h3_t.append(h3)

                # ---- P2: vertical sums via PE bands -> b (uint8, to DRAM) ----
                for s in range(S):
                    r0 = s * 128
                    sb = ps.tile([128, W], F32, tag="sb")
                    nc.tensor.matmul(sb[:], T3, h2_t[s][:], start=True, stop=False)
                    if s > 0:
                        nc.tensor.matmul(sb[:], Uc, h2_t[s - 1][:], start=False, stop=False)
                    if s < S - 1:
                        nc.tensor.matmul(sb[:], Dc, h2_t[s + 1][:], start=False, stop=False)
                    nc.tensor.matmul(sb[:], T2, h3_t[s][:], start=False, stop=(s == 0))
                    if s > 0:
                        nc.tensor.matmul(sb[:], Uc, h3_t[s - 1][:], start=False, stop=True)
                    b = pt.tile([128, W], U8, tag="b")
                    nc.vector.tensor_scalar(out=b[:], in0=sb[:], scalar1=0.5,
                                            scalar2=None, op0=OP.is_gt)
                    nc.sync.dma_start(bm[img, r0:r0 + 128, :], b[:])

                # ---- P3a: x load, exp, plane sums ----
                e_t = []
                for s in range(S):
                    r0 = s * 128
                    x = px.tile([128, C * W], BF16, tag="x")
                    for c in range(C):
                        nc.sync.dma_start(x[:, c * W:(c + 1) * W],
                                          xl[img, c, r0:r0 + 128, :])
                    e = px.tile([128, C * W], BF16, tag="e")
                    nc.scalar.activation(e[:], x[:], AF.Exp)
                    se = ps.tile([128, W], F32, tag="se")
                    for c in range(C):
                        nc.tensor.matmul(se[:], Ic, e[:, c * W:(c + 1) * W],
                                         start=(c == 0), stop=(c == C - 1))
                    e_t.append((x, se))

                # ---- P3b: ln + x_t class sums ----
                for s in range(S):
                    j = img * S + s
                    base = j * COLS_PER_STRIP
                    x, se = e_t[s]
                    lse = pt.tile([128, W], F32, tag="lse")
                    nc.scalar.activation(lse[:], se[:], AF.Ln,
                                         accum_out=cols[:, base + 0:base + 1])
                    scr2 = pt.tile([128, W], BF16, tag="scr2")
                    for c in range(C):
                        nc.vector.scalar_tensor_tensor(
                            out=scr2[:], in0=t_t[s][:], scalar=float(c),
                            in1=x[:, c * W:(c + 1) * W], op0=OP.is_equal, op1=OP.mult,
                            accum_out=cols[:, base + 1 + c:base + 2 + c])

            nc.sync.dma_start(out[:], cols[:])

    if split:
        _split_sync_waits(nc)
    _NC_CACHE[key] = nc
    return nc


def _host_reduce(results, x=None, t=None):
    """Assemble the loss from per-core accumulators + boundary maps.
    Returns (loss, ok); ok=False -> caller must run the exact fallback."""
    tot_lse = tot_xt = 0.0
    nb_idx = []   # (global_img, row, col) of non-boundary pixels
    for core, r in enumerate(results):
        cols = r["out"].astype(np.float64)
        for j in range(NSTRIP):
            base = j * COLS_PER_STRIP
            tot_lse += cols[:, base + 0].sum()
            tot_xt += cols[:, base + 1:base + 9].sum()
        bmap = r["bm"]
        nz = np.argwhere(bmap == 0)
        for (ii, rr, cc) in nz:
            nb_idx.append((core * PER + int(ii), int(rr), int(cc)))
            if len(nb_idx) >= 9:
                return 0.0, False
    s_ce = tot_lse - tot_xt
    corr = 0.0
    if nb_idx and x is not None:
        for (gi, rr, cc) in nb_idx:
            v = x[gi, :, rr, cc].astype(np.float64)
            lse = math.log(np.exp(v).sum())
            corr += lse - v[int(t[gi, rr, cc])]
    loss = (s_ce - (1.0 - C1) * corr) / NPIX
    return loss, True


def _pool3(a, op):
    pad = -np.inf if op is np.maximum else np.inf
    p = np.pad(a, ((0, 0), (1, 1), (1, 1)), constant_values=pad)
    r = a.copy()
    for dy in (-1, 0, 1):
        for dx in (-1, 0, 1):
            r = op(r, p[:, 1 + dy:H + 1 + dy, 1 + dx:W + 1 + dx])
    return r


def _fallback(x, t):
    """Exact numpy port of the reference (any input). Only taken when >=9
    non-boundary pixels exist (never for random multi-class targets)."""
    tf = t.astype(np.float32)
    bnd = (_pool3(tf, np.maximum) != _pool3(tf, np.minimum)).astype(np.float32)
    dist = np.zeros_like(bnd)
    cur = bnd.copy()
    for i in range(MAX_ITERS):
        dil = _pool3(cur, np.maximum)
        dist += (dil > cur).astype(np.float32) * (i + 1)
        cur = dil
    wts = np.exp(-dist / THETA)
    xm = x.max(axis=1, keepdims=True)
    lse = np.log(np.exp(x - xm).sum(axis=1)) + xm[:, 0]
    xt = np.take_along_axis(x, t[:, None].astype(np.int64), axis=1)[:, 0]
    return np.float32(np.mean((wts * (lse - xt)).astype(np.float64)))


def kernel(inputs, targets):
    x = np.ascontiguousarray(np.asarray(inputs))
    t = np.asarray(targets)
    xb = x.astype(ml_dtypes.bfloat16)
    tb = t.astype(ml_dtypes.bfloat16)
    cst = _band_consts()

    nc = _build_nc()
    in_maps = [
        {"xl": xb[i * PER:(i + 1) * PER], "tg": tb[i * PER:(i + 1) * PER], "cst": cst}
        for i in range(N_CORES)
    ]
    res = run_bass_kernel_spmd(nc, in_maps, list(range(N_CORES)))
    loss, ok = _host_reduce(res.results, x, t)
    if not ok:
        return _fallback(x, t)
    return np.float32(loss)


# revision 14
# speedup vs baseline: 28312.8448x; 1.5652x over previous
"""BoundaryLoss Trainium2 kernel (8-core data-parallel).

Math: boundary b[p] = 1 iff the 3x3 window around p spans >1 class.  The
reference's capped iterative distance transform assigns dist=0 to boundary
pixels, dist=D (chebyshev distance to the boundary) for 1<=D<=15, dist=0
beyond.  A pixel with D>=2 requires a fully non-boundary 3x3 block, i.e. at
least 9 non-boundary pixels in the image set; when the total non-boundary
count is < 9 (always, for random multi-class targets), every non-boundary
pixel has D==1 and the weights collapse to  w = c1 + (1-c1)*b,
c1 = exp(-1/theta).  Then

  loss * N = sum(ce) - (1-c1) * sum_{b==0}(ce),   ce = lse - x_t

The correction term touches <9 pixels; the host computes it exactly in f64
from the device-produced boundary map.  If the screen fails (>=9
non-boundary pixels) the host falls back to an exact numpy reference port.

Device layout: whole images free-stacked as [128 partitions, 4*512] tiles
(image row r = strip*128 + partition; strip lives in the free dim), so
per-pixel ops run image-at-a-time with multi-dim access patterns:
  - eh/ev: class-difference indicators (DVE tensor_tensor, bf16, 2x mode)
  - 3x3 window sums: horizontal adds on DVE; vertical via banded-matrix
    matmuls (T3/T2 + cross-strip halo bands) accumulated in PSUM (PE)
  - b = sum > 0 (DVE is_gt -> uint8), DMA'd out per image
  - CE: exp per class-plane on ACT (bf16); plane sum via identity-matmul
    PSUM accumulation (PE); lse = Ln on ACT with free-dim sum accumulator
  - sum(x_t): per-class scalar_tensor_tensor (t==c)*x_c with row-sum
    accumulator (DVE)
Host: sums accumulator columns in f64, applies the sparse correction.
"""
import math
import numpy as np
import ml_dtypes
import concourse.bass as bass
import concourse.tile as tile
from concourse import mybir
from concourse.bass_utils import run_bass_kernel_spmd

BF16 = mybir.dt.bfloat16
F32 = mybir.dt.float32
U8 = mybir.dt.uint8
AF = mybir.ActivationFunctionType
OP = mybir.AluOpType

B, C, H, W = 16, 8, 512, 512
N_CORES = 8
PER = B // N_CORES            # images per core
S = H // 128                  # strips per image
SW = S * W                    # stacked free width (2048)
THETA = 5.0
MAX_ITERS = 15
C1 = math.exp(-1.0 / THETA)
NPIX = B * H * W

# accumulator columns per image: 4 lse (per strip) + 8 xt (per class)
COLS_PER_IMG = S + C
NCOLS = PER * COLS_PER_IMG


def _split_sync_waits(nc, max_waits=1):
    """Walrus CoreV3 codegen rejects >1 sync wait per instruction; hoist
    extras onto NoOps inserted just before."""
    k = 0
    for f in nc.m.functions:
        for bb in f.blocks:
            new = []
            for ins in bb.instructions:
                w = list(ins.sync_info.on_wait) if ins.sync_info else []
                if len(w) > max_waits:
                    extra, keep = w[:-max_waits], w[-max_waits:]
                    for s0 in range(0, len(extra), max_waits):
                        nop = mybir.InstNoOp(
                            name=f"I-wsplit-{k}", ins=[], outs=[],
                            sync_info=mybir.SyncInfo(
                                on_wait=extra[s0:s0 + max_waits], on_update=[]),
                            engine=ins.engine)
                        k += 1
                        new.append(nop)
                    ins.sync_info.on_wait = keep
                new.append(ins)
            bb.instructions = new


def _band_consts():
    """bf16 [128, 5*128]: T3 (tridiag), T2 (k in {p-1,p}), U (k=127 -> p=0),
    D (k=0 -> p=127), I (identity). lhsT layout: [k, p]."""
    k = np.arange(128)[:, None]
    p = np.arange(128)[None, :]
    T3 = (np.abs(k - p) <= 1).astype(np.float32)
    T2 = ((k == p) | (k == p - 1)).astype(np.float32)
    U = ((k == 127) & (p == 0)).astype(np.float32)
    D = ((k == 0) & (p == 127)).astype(np.float32)
    I = (k == p).astype(np.float32)
    return np.concatenate([T3, T2, U, D, I], axis=1).astype(ml_dtypes.bfloat16)


_NC_CACHE = {}


def _blk(ap):
    """[128, S*W] -> [128, S, W] view."""
    return ap.rearrange("p (s w) -> p s w", s=S)


def _stk(dram_img):
    """DRAM [H, W] -> [128, S, W] view matching the stacked SBUF layout."""
    return dram_img.rearrange("(s p) w -> p s w", p=128)


def _build_nc(repeat=1, split=True):
    """repeat>1 re-runs the whole per-core computation, overwriting the same
    accumulators -- output equals the repeat=1 result; used for timing."""
    key = (repeat, split)
    if key in _NC_CACHE:
        return _NC_CACHE[key]
    nc = bass.Bass()
    xl = nc.dram_tensor("xl", [PER, C, H, W], BF16, kind="ExternalInput")
    tg = nc.dram_tensor("tg", [PER, H, W], BF16, kind="ExternalInput")
    cst = nc.dram_tensor("cst", [128, 5 * 128], BF16, kind="ExternalInput")
    out = nc.dram_tensor("out", [128, NCOLS], F32, kind="ExternalOutput")
    bm = nc.dram_tensor("bm", [PER, H, W], U8, kind="ExternalOutput")

    with tile.TileContext(nc) as tc:
        with (
            tc.tile_pool(name="pc", bufs=1) as pc,
            tc.tile_pool(name="pp", bufs=2) as pp,      # per-image maps
            tc.tile_pool(name="pt", bufs=2) as pt,      # transients
            tc.tile_pool(name="px", bufs=2) as px,      # big x/e tiles
            tc.tile_pool(name="pa", bufs=1) as pa,      # accumulator columns
            tc.tile_pool(name="ps", bufs=2, space="PSUM") as ps,
            tc.tile_pool(name="ps1", bufs=1, space="PSUM") as ps1,
        ):
            cons = pc.tile([128, 5 * 128], BF16, tag="cons")
            nc.sync.dma_start(cons[:], cst[:])
            T3 = cons[:, 0:128]
            T2 = cons[:, 128:256]
            Uc = cons[:, 256:384]
            Dc = cons[:, 384:512]
            Ic = cons[:, 512:640]

            cols = pa.tile([128, NCOLS], F32, tag="cols")

            for rep_i, img in enumerate(i % PER for i in range(repeat * PER)):
                # ---- t loads (stacked) ----
                t = pp.tile([128, SW], BF16, tag="t")
                nc.gpsimd.dma_start(_blk(t[:]), _stk(tg[img]))
                td = pt.tile([128, SW], BF16, tag="td")
                for s in range(S):
                    r0 = s * 128
                    if s < S - 1:
                        nc.gpsimd.dma_start(td[:, s * W:(s + 1) * W],
                                            tg[img, r0 + 1:r0 + 129, :])
                    else:
                        nc.gpsimd.dma_start(td[0:127, s * W:(s + 1) * W],
                                            tg[img, r0 + 1:r0 + 128, :])
                        nc.gpsimd.dma_start(td[127:128, s * W:(s + 1) * W],
                                            tg[img, H - 1:H, :])

                tb, tdb = _blk(t[:]), _blk(td[:])
                # ---- edge maps ----
                # eh[s][c] = t[s][c] != t[s][c+1] (c<511); col 511 = 0
                eh = pt.tile([128, SW], BF16, tag="eh")
                ehb = _blk(eh[:])
                nc.gpsimd.memset(ehb[:, :, W - 1:W], 0.0)
                nc.vector.tensor_tensor(out=ehb[:, :, 0:W - 1], in0=tb[:, :, 0:W - 1],
                                        in1=tb[:, :, 1:W], op=OP.not_equal)
                # ev = t != t_down (last image row clamps -> 0)
                ev = pt.tile([128, SW], BF16, tag="ev")
                nc.vector.tensor_tensor(out=ev[:], in0=t[:], in1=td[:],
                                        op=OP.not_equal)
                evb = _blk(ev[:])
                # H2eh[c] = eh[c-1] + eh[c]
                h2 = pp.tile([128, SW], BF16, tag="h2")
                h2b = _blk(h2[:])
                nc.gpsimd.tensor_copy(h2b[:, :, 0:1], ehb[:, :, 0:1])
                nc.vector.tensor_tensor(out=h2b[:, :, 1:W], in0=ehb[:, :, 0:W - 1],
                                        in1=ehb[:, :, 1:W], op=OP.add)
                # H3ev[c] = ev[c-1] + ev[c] + ev[c+1]
                h3 = pp.tile([128, SW], BF16, tag="h3")
                h3b = _blk(h3[:])
                tmp = pt.tile([128, SW], BF16, tag="tmp")
                tmpb = _blk(tmp[:])
                nc.vector.tensor_tensor(out=tmpb[:, :, 0:W - 1], in0=evb[:, :, 0:W - 1],
                                        in1=evb[:, :, 1:W], op=OP.add)
                nc.vector.tensor_tensor(out=h3b[:, :, 1:W - 1], in0=tmpb[:, :, 0:W - 2],
                                        in1=evb[:, :, 2:W], op=OP.add)
                nc.gpsimd.tensor_copy(h3b[:, :, 0:1], tmpb[:, :, 0:1])
                nc.gpsimd.tensor_copy(h3b[:, :, W - 1:W], tmpb[:, :, W - 2:W - 1])

                # ---- vertical sums via PE bands -> b (uint8, stacked) ----
                bt = pt.tile([128, SW], U8, tag="bt")
                for s in range(S):
                    c0, c1_ = s * W, (s + 1) * W
                    sb = ps.tile([128, W], F32, tag="sb")
                    nc.tensor.matmul(sb[:], T3, h2[:, c0:c1_], start=True, stop=False)
                    if s > 0:
                        nc.tensor.matmul(sb[:], Uc, h2[:, c0 - W:c0], start=False, stop=False)
                    if s < S - 1:
                        nc.tensor.matmul(sb[:], Dc, h2[:, c1_:c1_ + W], start=False, stop=False)
                    nc.tensor.matmul(sb[:], T2, h3[:, c0:c1_], start=False, stop=(s == 0))
                    if s > 0:
                        nc.tensor.matmul(sb[:], Uc, h3[:, c0 - W:c0], start=False, stop=True)
                    nc.vector.tensor_scalar(out=bt[:, c0:c1_], in0=sb[:], scalar1=0.5,
                                            scalar2=None, op0=OP.is_gt)
                nc.sync.dma_start(_stk(bm[img]), _blk(bt[:]))

                # ---- CE: per class: x load -> exp -> x_t partial -> plane sums ----
                base = img * COLS_PER_IMG
                scr2 = pt.tile([128, SW], BF16, tag="scr2")
                se_t = [ps1.tile([128, W], F32, tag=f"se{s}", name=f"se{s}")
                        for s in range(S)]
                for c in range(C):
                    xc = px.tile([128, SW], BF16, tag="x")
                    nc.sync.dma_start(_blk(xc[:]), _stk(xl[img, c]))
                    ec = px.tile([128, SW], BF16, tag="e")
                    nc.scalar.activation(ec[:], xc[:], AF.Exp)
                    nc.vector.scalar_tensor_tensor(
                        out=scr2[:], in0=t[:], scalar=float(c),
                        in1=xc[:], op0=OP.is_equal, op1=OP.mult,
                        accum_out=cols[:, base + S + c:base + S + c + 1])
                    for s in range(S):
                        nc.tensor.matmul(se_t[s][:], Ic, ec[:, s * W:(s + 1) * W],
                                         start=(c == 0), stop=(c == C - 1))

                # ---- ln per strip ----
                lse = pt.tile([128, SW], F32, tag="lse")
                for s in range(S):
                    nc.scalar.activation(lse[:, s * W:(s + 1) * W], se_t[s][:], AF.Ln,
                                         accum_out=cols[:, base + s:base + s + 1])

            nc.sync.dma_start(out[:], cols[:])

    if split:
        _split_sync_waits(nc)
    _NC_CACHE[key] = nc
    return nc


def _host_reduce(results, x=None, t=None):
    """Assemble the loss from per-core accumulators + boundary maps.
    Returns (loss, ok); ok=False -> caller must run the exact fallback."""
    nb_idx = []   # (global_img, row, col) of non-boundary pixels
    tot_lse = tot_xt = 0.0
    for core, r in enumerate(results):
        bmap = r["bm"]
        for (ii, rr, cc) in np.argwhere(bmap == 0):
            nb_idx.append((core * PER + int(ii), int(rr), int(cc)))
            if len(nb_idx) >= 9:
                return 0.0, False
        cols = r["out"].astype(np.float64)
        for img in range(PER):
            base = img * COLS_PER_IMG
            tot_lse += cols[:, base:base + S].sum()
            tot_xt += cols[:, base + S:base + S + C].sum()
    s_ce = tot_lse - tot_xt
    corr = 0.0
    if nb_idx and x is not None:
        for (gi, rr, cc) in nb_idx:
            v = x[gi, :, rr, cc].astype(np.float64)
            lse = math.log(np.exp(v).sum())
            corr += lse - v[int(t[gi, rr, cc])]
    loss = (s_ce - (1.0 - C1) * corr) / NPIX
    return loss, True


def _pool3(a, op):
    pad = -np.inf if op is np.maximum else np.inf
    p = np.pad(a, ((0, 0), (1, 1), (1, 1)), constant_values=pad)
    r = a.copy()
    for dy in (-1, 0, 1):
        for dx in (-1, 0, 1):
            r = op(r, p[:, 1 + dy:H + 1 + dy, 1 + dx:W + 1 + dx])
    return r


def _fallback(x, t):
    """Exact numpy port of the reference (any input). Only taken when >=9
    non-boundary pixels exist (never for random multi-class targets)."""
    tf = t.astype(np.float32)
    bnd = (_pool3(tf, np.maximum) != _pool3(tf, np.minimum)).astype(np.float32)
    dist = np.zeros_like(bnd)
    cur = bnd.copy()
    for i in range(MAX_ITERS):
        dil = _pool3(cur, np.maximum)
        dist += (dil > cur).astype(np.float32) * (i + 1)
        cur = dil
    wts = np.exp(-dist / THETA)
    xm = x.max(axis=1, keepdims=True)
    lse = np.log(np.exp(x - xm).sum(axis=1)) + xm[:, 0]
    xt = np.take_along_axis(x, t[:, None].astype(np.int64), axis=1)[:, 0]
    return np.float32(np.mean((wts * (lse - xt)).astype(np.float64)))


def kernel(inputs, targets):
    x = np.ascontiguousarray(np.asarray(inputs))
    t = np.asarray(targets)
    xb = x.astype(ml_dtypes.bfloat16)
    tb = t.astype(ml_dtypes.bfloat16)
    cst = _band_consts()

    nc = _build_nc()
    in_maps = [
        {"xl": xb[i * PER:(i + 1) * PER], "tg": tb[i * PER:(i + 1) * PER], "cst": cst}
        for i in range(N_CORES)
    ]
    res = run_bass_kernel_spmd(nc, in_maps, list(range(N_CORES)))
    loss, ok = _host_reduce(res.results, x, t)
    if not ok:
        return _fallback(x, t)
    return np.float32(loss)
